# revision 25
# baseline (speedup 1.0000x reference)
"""Trainium2 Bass kernel for nn_MistralMoLoraLayer (MoE-routed LoRA FFN).

Strategy: data-parallel over tokens (8 cores x 256 tokens), base FFN weights
replicated. The per-(batch,slot) softmax over the sequence axis needs global
denominators -> tiny [128,4] AllReduce whose result is consumed ONLY by
GpSimd/late ops, so the eager PE stream never waits on the collective.

fp8 DoubleRow scheme (cost model charges DR matmuls 0.5 cycles/row with
K=256 per instruction): the big GEMMs (base up/gate, down) run as e4m3
hi/lo 3-chain DR matmuls -- x = xh + xl, w = wh + wl, R = xh@wh + xh@wl +
xl@wh -- which carries ~8 mantissa bits (bf16-level noise, measured 2.1e-3
end-to-end on CPU emulation) at 0.75x the bf16 PE cost. LoRA B matmuls run
single-pass fp8 (noise ~measured 9.6e-3 max-rel, inside the 2e-2 gate).
Intermediates (tu/tg/su/hh/mixed/vt) are fp16 instead of bf16 (same DVE/PE
cost, 8x less rounding noise).

Scale plumbing (all powers of 2, folded into existing ops):
  x*SX (8), wu/wg*SWU (32) -> psUG at SS=256; Ut*SU(16), uB*SB(16) ->
  psLO also at 256 -> tu/tg at 256 in PSUM; silu descales via the Act
  scale input; hh = su * tg_scaled carries SS -> mixed lands at SS=256
  (the fp8 quantization scale for the down moving operand, no extra op);
  wd*SWD (64), dB*SWD -> psO at SS*SWD=16384, descaled in the output copy.

Per-core math (tiles [h/er/d partitions, tokens free]):
  router: logits = x @ gate_w.T to ~fp24 via bf16 hi/lo split chains;
          top-2 per token; exp; AR of per-batch-slot sums; weights from AR
  A-proj: UA/GA [E*R=128, t] = stacked up_A/gate_A @ x.T (bf16)
  slot-mask: Ut8_j = fp8(UA*16 * M_j); lo_up_j = (up_B8) @ Ut8_j
  h_j = silu(U + lo_up_j) * (G + lo_gate_j)     (fp16, scaled by 256)
  mixed = c_0*h_0 + c_1*h_1 (Pool, AR-deferred) -> fp8 hi/lo for down DR
  v[er,t] = (stacked down_A fp16) @ hh_pair      (accumulated over h)
  outT[d,t] = wd-DR-chains @ mixed_hi/lo + dB @ (masked weighted v)
"""

import numpy as np

# problem constants (hardcoded; kernel.py must be self-contained)
B, S, D, H, E, R, TOPK = 2, 1024, 2048, 5632, 8, 16, 2
ALPHA = 2.0
T = B * S
NCORES = 8
TC = T // NCORES           # 256 tokens per core
KT = D // 128              # 16 k-tiles over D
HT = H // 128              # 44 h-tiles
DT = D // 128              # 16 d-tiles
ER = E * R                 # 128

SX = 8.0                   # x fp8 scale
SWU = 32.0                 # w_up/w_gate fp8 scale
SS = SX * SWU              # 256: psUG/psLO/hh/mixed scale
SU = 16.0                  # Ut fp8 scale
SB = SS / SU               # 16: up_B/gate_B fp8 scale
SWD = 64.0                 # w_down fp8 scale (psO at SS*SWD)

DEBUG_TAPS = False         # add intermediate-tensor outputs for debugging
SKIP_AR = False            # replace AllReduce with local copy (for TimelineSim)

_cache = {}


def _build():
    import concourse.bacc as bacc
    import concourse.bass as bass
    import concourse.mybir as mybir
    import concourse.tile as tile
    from concourse.masks import make_identity

    f32 = mybir.dt.float32
    bf16 = mybir.dt.bfloat16
    fp16 = mybir.dt.float16
    fp8 = mybir.dt.float8e4
    AL = mybir.AluOpType
    AF = mybir.ActivationFunctionType
    DR = mybir.MatmulPerfMode.DoubleRow

    nc = bacc.Bacc("TRN2", target_bir_lowering=False, debug=False,
                   num_devices=NCORES)

    # ---- DRAM I/O (host-prepped layouts; single-DMA where possible) ----
    # x ships as bf16 hi + bf16 lo (x = hi + lo to ~fp24) for the router,
    # plus e4m3 hi/lo (x*SX) for the base-GEMM DoubleRow chains
    d_xTb = nc.dram_tensor("xTb", [128, KT * TC], bf16, kind="ExternalInput").ap()
    d_xlo = nc.dram_tensor("xlo", [128, KT * TC], bf16, kind="ExternalInput").ap()
    d_xh8 = nc.dram_tensor("xh8", [128, KT, TC], fp8, kind="ExternalInput").ap()
    d_xl8 = nc.dram_tensor("xl8", [128, KT, TC], fp8, kind="ExternalInput").ap()
    # cols 0:KT*E = gate_w hi, KT*E:2*KT*E = gate_w lo
    d_gwp = nc.dram_tensor("gwp", [128, 2 * KT * E], bf16, kind="ExternalInput").ap()
    # per h-tile: [2*KT, 128] fp8: rows 0:KT = hi k-tiles, KT:2KT = lo
    d_wu = nc.dram_tensor("wu", [HT, 128, 2 * KT, 128], fp8, kind="ExternalInput").ap()
    d_wg = nc.dram_tensor("wg", [HT, 128, 2 * KT, 128], fp8, kind="ExternalInput").ap()
    d_wdh = nc.dram_tensor("wdh", [DT, 128, HT, 128], fp8, kind="ExternalInput").ap()
    d_wdl = nc.dram_tensor("wdl", [DT, 128, HT, 128], fp8, kind="ExternalInput").ap()
    d_A = nc.dram_tensor("Ah", [128, KT * 2 * ER], bf16, kind="ExternalInput").ap()
    # per-h-tile LoRA pack (fp8 bytes): 0:128 up_B8, 128:256 gate_B8,
    # 256:512 down_A as fp16 bytes (bitcast on chip)
    d_Bp = nc.dram_tensor("Bp", [HT, 128, 512], fp8, kind="ExternalInput").ap()
    d_dB = nc.dram_tensor("dB", [128, D], fp16, kind="ExternalInput").ap()
    # col 0 = eid, cols 1:9 = i8m
    d_ei = nc.dram_tensor("ei", [128, 1 + E], f32, kind="ExternalInput").ap()
    # bscat[s, 2b+s] = 1 for this core's batch b; bmask = 1 at cols 2b,2b+1
    d_bscat = nc.dram_tensor("bscat", [2, 4], f32, kind="ExternalInput").ap()
    d_bmask = nc.dram_tensor("bmask", [128, 4], f32, kind="ExternalInput").ap()
    d_sel2 = nc.dram_tensor("sel2", [2, 256], f32, kind="ExternalInput").ap()
    d_out = nc.dram_tensor("outT", [D, TC], bf16, kind="ExternalOutput").ap()

    with tile.TileContext(nc) as tc:
        import contextlib
        ctx = contextlib.ExitStack()
        with ctx:
            cpool = ctx.enter_context(tc.tile_pool(name="const", bufs=1))
            wpool = ctx.enter_context(tc.tile_pool(name="wstream", bufs=2))
            bpool = ctx.enter_context(tc.tile_pool(name="bstream", bufs=3))
            spool = ctx.enter_context(tc.tile_pool(name="work", bufs=2))
            pspool = ctx.enter_context(
                tc.tile_pool(name="ps", bufs=1, space="PSUM"))
            drpool = ctx.enter_context(
                tc.tile_pool(name="dram", bufs=1, space="DRAM"))

            # ---- DMA plan: SP queue carries x + the wu/wg stream; the
            # Activation queue carries everything else ----
            xTb = wpool.tile([128, KT * TC], bf16, tag="xw", bufs=2,
                             name="xTb")
            nc.sync.dma_start(out=xTb[:], in_=d_xTb[:])
            xlo = wpool.tile([128, KT * TC], bf16, tag="xw", bufs=2,
                             name="xlo")
            nc.sync.dma_start(out=xlo[:], in_=d_xlo[:])
            xh8 = cpool.tile([128, KT, TC], fp8, name="xh8")
            nc.sync.dma_start(out=xh8[:], in_=d_xh8[:])
            xl8 = cpool.tile([128, KT, TC], fp8, name="xl8")
            nc.sync.dma_start(out=xl8[:], in_=d_xl8[:])

            # warm both activation tables first thing on the Act engine (Silu
            # then Exp, so Exp is resident for the router; Silu reloads once
            # at h-loop start, off the AR critical path)
            warm = cpool.tile([1, 2], f32, name="warm")
            nc.vector.memset(warm, 0.0)
            nc.scalar.activation(warm[:, 0:1], warm[:, 0:1], AF.Silu)
            nc.scalar.activation(warm[:, 1:2], warm[:, 1:2], AF.Exp)

            gwp_sb = cpool.tile([128, 2 * KT * E], bf16, name="gwp_sb")
            nc.scalar.dma_start(out=gwp_sb[:], in_=d_gwp[:])
            A_sb = cpool.tile([128, KT * 2 * ER], bf16, name="A_sb")
            AG0 = KT * ER
            nc.scalar.dma_start(out=A_sb[:, 0:AG0], in_=d_A[:, 0:AG0])
            nc.scalar.dma_start(out=A_sb[:, AG0:2 * AG0], in_=d_A[:, AG0:2 * AG0])
            ei_sb = cpool.tile([128, 1 + E], f32, name="ei_sb")
            nc.scalar.dma_start(out=ei_sb[:], in_=d_ei[:])
            eid_sb = ei_sb[:, 0:1]
            i8m_sb = ei_sb[:, 1:1 + E]
            bscat_sb = cpool.tile([2, 4], f32, name="bscat_sb")
            nc.scalar.dma_start(out=bscat_sb[:], in_=d_bscat[:])
            bmask_sb = cpool.tile([128, 4], f32, name="bmask_sb")
            nc.scalar.dma_start(out=bmask_sb[:], in_=d_bmask[:])
            sel2_sb = cpool.tile([2, 256], f32, name="sel2_sb")
            nc.scalar.dma_start(out=sel2_sb[:], in_=d_sel2[:])
            dB_sb = cpool.tile([128, D], fp16, name="dB_sb")
            nc.scalar.dma_start(out=dB_sb[:], in_=d_dB[:])

            ident = cpool.tile([128, 128], f32, name="ident")
            make_identity(nc, ident)
            ones2 = cpool.tile([2, 128], f32, name="ones2")
            nc.vector.memset(ones2, 1.0)

            # fp8 hi/lo of mixed (at scale SS), the down-GEMM moving operand
            mixed_h8 = cpool.tile([128, HT, TC], fp8, name="mixed_h8")
            mixed_l8 = cpool.tile([128, HT, TC], fp8, name="mixed_l8")
            ev_rows = cpool.tile([2, TC], f32, name="ev_rows")
            s_rows = cpool.tile([2, TC], f32, name="s_rows")
            cbraw = cpool.tile([128, 2 * TC], f32, name="cbraw")
            cb = cpool.tile([128, 2 * TC], fp16, name="cb")
            Mj = cpool.tile([128, 2 * TC], f32, name="Mj")
            UA = cpool.tile([128, TC], f32, name="UA")
            GA = cpool.tile([128, TC], f32, name="GA")
            Ut8 = cpool.tile([128, 2 * TC], fp8, name="Ut8")
            Gt8 = cpool.tile([128, 2 * TC], fp8, name="Gt8")
            vt = cpool.tile([128, 2 * TC], fp16, name="vt")

            # ---- phase 1a: router logit chains (PE, fp24 via hi/lo) ----
            GWH = KT * E
            psL = {}
            for tt in range(2):
                psL[tt] = pspool.tile([128, TC], f32, tag="psUG", bufs=3,
                                      name=f"psL{tt}")
                passes = [(xTb, 0), (xTb, GWH), (xlo, 0)]
                for pi, (xa, go) in enumerate(passes):
                    for k in range(KT):
                        nc.tensor.matmul(
                            psL[tt][:, 0:E],
                            xa[:, k * TC + tt * 128: k * TC + tt * 128 + 128],
                            gwp_sb[:, go + k * E:go + (k + 1) * E],
                            start=(pi == 0 and k == 0),
                            stop=(pi == 2 and k == KT - 1))

            # ---- phase 0: stacked A-projections (bf16) ----
            psUA = pspool.tile([128, TC], f32, tag="psV", name="psUA")
            for k in range(KT):
                nc.tensor.matmul(psUA[:],
                                 A_sb[:, k * ER:(k + 1) * ER],
                                 xTb[:, k * TC:(k + 1) * TC],
                                 start=(k == 0), stop=(k == KT - 1))
            nc.scalar.activation(UA[:], psUA[:], AF.Copy, scale=SU)
            psGA = pspool.tile([128, TC], f32, tag="psV", name="psGA")
            for k in range(KT):
                nc.tensor.matmul(psGA[:],
                                 A_sb[:, AG0 + k * ER:AG0 + (k + 1) * ER],
                                 xTb[:, k * TC:(k + 1) * TC],
                                 start=(k == 0), stop=(k == KT - 1))
            nc.scalar.activation(GA[:], psGA[:], AF.Copy, scale=SU)

            # ---- phase 1b: top-2 select + exp (both token-tiles overlap) ----
            evs, svs = {}, {}
            for tt in range(2):
                L = spool.tile([128, E], f32, tag="L")
                nc.vector.tensor_copy(L[:], psL[tt][:, 0:E])
                mx1 = spool.tile([128, 1], f32, tag="mx1")
                nc.vector.tensor_reduce(mx1[:], L[:], mybir.AxisListType.X, AL.max)
                msk = spool.tile([128, E], f32, tag="msk")
                nc.vector.tensor_scalar(msk[:], L[:], mx1[:], None, AL.is_equal)
                mi = spool.tile([128, E], f32, tag="mi")
                nc.vector.tensor_tensor(mi[:], msk[:], i8m_sb[:], AL.mult)
                svals = spool.tile([128, 2], f32, tag="svals")
                nc.vector.tensor_reduce(svals[:, 0:1], mi[:],
                                        mybir.AxisListType.X, AL.max)
                evals = spool.tile([128, 2], f32, tag="evals")
                nc.scalar.activation(evals[:, 0:1], mx1[:], AF.Exp)
                # mask out slot-0 winner, find second max
                big = spool.tile([128, E], f32, tag="big")
                nc.vector.tensor_scalar(big[:], msk[:], 1e30, None, AL.mult)
                L2 = spool.tile([128, E], f32, tag="L2")
                nc.vector.tensor_tensor(L2[:], L[:], big[:], AL.subtract)
                mx2 = spool.tile([128, 1], f32, tag="mx2")
                nc.vector.tensor_reduce(mx2[:], L2[:], mybir.AxisListType.X, AL.max)
                msk2 = spool.tile([128, E], f32, tag="msk2")
                nc.vector.tensor_scalar(msk2[:], L2[:], mx2[:], None, AL.is_equal)
                mi2 = spool.tile([128, E], f32, tag="mi2")
                nc.vector.tensor_tensor(mi2[:], msk2[:], i8m_sb[:], AL.mult)
                nc.vector.tensor_reduce(svals[:, 1:2], mi2[:],
                                        mybir.AxisListType.X, AL.max)
                nc.scalar.activation(evals[:, 1:2], mx2[:], AF.Exp)
                evs[tt], svs[tt] = evals, svals

            # ---- phases 2+5+6: h-tile loop ----
            psV = pspool.tile([128, 2 * TC], f32, tag="psV", name="psV")
            pend_v = []                 # delayed psV matmuls [(dA_t, hh_pair)]
            PEND = 3                    # psV deferred this many h-tiles
            CBL = 12                    # cb application deferred this many
            ch_defer = []               # (hh_pair, i)
            psv_started = [False]

            def flush_v(last=False):
                while pend_v and (last or len(pend_v) > PEND):
                    pv_dA, pv_hh = pend_v.pop(0)
                    stop = last and not pend_v
                    nc.tensor.matmul(psV[:], pv_dA, pv_hh[:],
                                     start=not psv_started[0], stop=stop,
                                     skip_group_check=True)
                    psv_started[0] = True

            wd_pre = {}

            def load_wd(di, tag="wd", bufs=11):
                # wd streams split across the SP and Act queues (per-queue
                # issue chains serialize on DMA completions); the first pair
                # reuses the router-x buffers (tag "xw"), dead by then
                th = wpool.tile([128, HT, 128], fp8, tag=tag, bufs=bufs)
                nc.sync.dma_start(out=th[:], in_=d_wdh[di])
                tl_ = wpool.tile([128, HT, 128], fp8, tag=tag, bufs=bufs)
                nc.scalar.dma_start(out=tl_[:], in_=d_wdl[di])
                wd_pre[di] = (th, tl_)

            def flush_mixed(drain=False):
                # post-AR elementwise ops: in-loop, the hh*cb mults live on
                # the otherwise-idle Pool queue so the eager DVE/Act/PE
                # streams never block on the AllReduce; the post-loop DRAIN
                # flushes run on DVE (idle by then, and the AR completed
                # long ago) so the down phase isn't Pool-serialized.
                # mh8 on Act, ml8 on DVE to balance engine load.
                f_hh, fi = ch_defer.pop(0)
                eng = (nc.vector if fi % 2 else nc.gpsimd) if drain \
                    else nc.gpsimd
                cp = spool.tile([128, 2 * TC], fp16, tag="ct")
                eng.tensor_tensor(cp[:], f_hh[:], cb[:], AL.mult)
                m = spool.tile([128, TC], fp16, tag="mt", bufs=3)
                eng.tensor_tensor(m[:], cp[:, 0:TC], cp[:, TC:2 * TC],
                                  AL.add)
                nc.scalar.activation(mixed_h8[:, fi, :], m[:], AF.Copy)
                nc.vector.tensor_tensor(mixed_l8[:, fi, :], m[:],
                                        mixed_h8[:, fi, :], AL.subtract)

            for i in range(HT):
                wu_t = wpool.tile([128, 2 * KT, 128], fp8, tag="wu", bufs=5)
                nc.sync.dma_start(out=wu_t[:], in_=d_wu[i])
                wg_t = wpool.tile([128, 2 * KT, 128], fp8, tag="wg", bufs=5)
                nc.scalar.dma_start(out=wg_t[:], in_=d_wg[i])
                Bp_t = bpool.tile([128, 512], fp8, tag="Bp", bufs=10)
                nc.scalar.dma_start(out=Bp_t[:], in_=d_Bp[i])
                uB_t = Bp_t[:, 0:128]
                gB_t = Bp_t[:, 128:256]
                dA_t = Bp_t[:, 256:512].bitcast(fp16)

                # base up/gate: e4m3 hi/lo 3-chain DoubleRow (K=256/instr)
                psUG = pspool.tile([128, 2 * TC], f32, tag="psUG", bufs=3,
                                   name="psUG")
                NK2 = KT // 2
                for col, w_t in ((0, wu_t), (TC, wg_t)):
                    chains = [(0, xh8), (KT, xh8), (0, xl8)]
                    for ci, (wo, xs) in enumerate(chains):
                        for kk in range(NK2):
                            nc.tensor.matmul(
                                psUG[:, col:col + TC],
                                w_t[:, wo + 2 * kk:wo + 2 * kk + 2, :],
                                xs[:, 2 * kk:2 * kk + 2, :],
                                start=(ci == 0 and kk == 0),
                                stop=(ci == 2 and kk == NK2 - 1),
                                perf_mode=DR)
                if i == 0:
                    # router epilogue + AR issue + masks, emitted after
                    # iter-0's base chains so the front-of-stream PE ops
                    # never wait on the vector chain
                    for tt in range(2):
                        evals, svals = evs[tt], svs[tt]
                        # transpose evals/svals -> rows
                        psT = pspool.tile([2, 128], f32, tag="psLO", bufs=2,
                                          name="psT")
                        nc.tensor.transpose(psT[:], evals[:], ident[:])
                        nc.vector.tensor_copy(ev_rows[:, tt * 128:(tt + 1) * 128], psT[:])
                        psT2 = pspool.tile([2, 128], f32, tag="psLO", bufs=2,
                                           name="psT2")
                        nc.tensor.transpose(psT2[:], svals[:], ident[:])
                        nc.vector.tensor_copy(s_rows[:, tt * 128:(tt + 1) * 128], psT2[:])

                    # partition-replicated AllReduce payload [128, 4] with columns
                    # (batch, slot): ar[p, 2b+s] = sum_t exp_s(t). The post-AR
                    # consumer is then pure DVE (no PE op ever waits on the AR).
                    denc = cpool.tile([2, 1], f32, name="denc")
                    nc.vector.tensor_reduce(denc[:], ev_rows[:], mybir.AxisListType.X,
                                            AL.add)
                    bden = cpool.tile([2, 4], f32, name="bden")
                    nc.vector.tensor_scalar(bden[:], bscat_sb[:], denc[:], None,
                                            AL.mult)
                    psA2 = pspool.tile([128, 4], f32, tag="psLO", bufs=2,
                                       name="psA2")
                    nc.tensor.matmul(psA2[:], ones2[:], bden[:], start=True, stop=True)
                    ar_sb = cpool.tile([128, 4], f32, name="ar_sb")
                    nc.vector.tensor_copy(ar_sb[:], psA2[:])
                    ar_in = drpool.tile([128, 4], f32, name="ar_in")
                    ar_out = drpool.tile([128, 4], f32, name="ar_out",
                                         addr_space="Shared")
                    nc.gpsimd.dma_start(out=ar_in[:], in_=ar_sb[:])
                    if SKIP_AR:
                        nc.gpsimd.dma_start(out=ar_out[:], in_=ar_in[:])
                    else:
                        nc.gpsimd.collective_compute(
                            "AllReduce", AL.add,
                            replica_groups=[list(range(NCORES))],
                            ins=[ar_in.opt()], outs=[ar_out.opt()])
                    den2b = cpool.tile([128, 4], f32, name="den2b")
                    nc.gpsimd.dma_start(out=den2b[:], in_=ar_out[:])

                    # ---- AR-independent prep: masks, unnormalized weight rows ----
                    # broadcast slot rows along partitions via K=2 matmul with a
                    # row-selector constant (sel2[:, j*128:(j+1)*128] has row j = 1)
                    for j in range(2):
                        psM = pspool.tile([128, TC], f32, tag="psV", name="psM")
                        nc.tensor.matmul(psM[:], sel2_sb[:, j * 128:(j + 1) * 128],
                                         s_rows[:], start=True, stop=True)
                        nc.vector.tensor_scalar(Mj[:, j * TC:(j + 1) * TC], psM[:],
                                                eid_sb[:], None, AL.is_equal)
                        psB = pspool.tile([128, TC], f32, tag="psV", name="psB")
                        nc.tensor.matmul(psB[:], sel2_sb[:, j * 128:(j + 1) * 128],
                                         ev_rows[:], start=True, stop=True)
                        nc.vector.tensor_copy(cbraw[:, j * TC:(j + 1) * TC], psB[:])
                    for j in range(2):
                        nc.vector.tensor_tensor(Ut8[:, j * TC:(j + 1) * TC], UA[:],
                                                Mj[:, j * TC:(j + 1) * TC], AL.mult)
                        nc.vector.tensor_tensor(Gt8[:, j * TC:(j + 1) * TC], GA[:],
                                                Mj[:, j * TC:(j + 1) * TC], AL.mult)

                flush_v()

                psLO = pspool.tile([128, 4 * TC], f32, tag="psLO", bufs=2,
                                   name="psLO")
                # both slots per B matrix in ONE free=512 fp8 matmul (Ut8/Gt8
                # hold j0|j1 contiguously); emitted BEFORE any DVE consumer of
                # psLO so tile-granular WAR tracking can't stall the PE queue
                nc.tensor.matmul(psLO[:, 0:2 * TC], uB_t, Ut8[:],
                                 start=True, stop=True)
                nc.tensor.matmul(psLO[:, 2 * TC:4 * TC], gB_t, Gt8[:],
                                 start=True, stop=True)
                # base PSUM -> SBUF fp16 (a DVE op cannot read two PSUM
                # operands, so the lora add reads UG_sb + psLO instead)
                UG_sb = spool.tile([128, 2 * TC], fp16, tag="ugs")
                nc.scalar.activation(UG_sb[:], psUG[:], AF.Copy)
                # tu/tg = base + lora (both at scale SS); fp16 out
                tusg = spool.tile([128, 2 * TC], fp16, tag="tus")
                tgsg = spool.tile([128, 2 * TC], fp16, tag="tgs")
                for j in range(2):
                    nc.vector.tensor_tensor(
                        tusg[:, j * TC:(j + 1) * TC], UG_sb[:, 0:TC],
                        psLO[:, j * TC:(j + 1) * TC], AL.add)
                    nc.vector.tensor_tensor(
                        tgsg[:, j * TC:(j + 1) * TC], UG_sb[:, TC:2 * TC],
                        psLO[:, (2 + j) * TC:(3 + j) * TC], AL.add)
                # silu descales via the Act scale input; hh carries SS via tg
                sup = spool.tile([128, 2 * TC], fp16, tag="sup")
                nc.scalar.activation(sup[:], tusg[:], AF.Silu, scale=1.0 / SS)
                hh_pair = spool.tile([128, 2 * TC], fp16, tag="hhp", bufs=15)
                nc.vector.tensor_tensor(hh_pair[:], sup[:], tgsg[:], AL.mult)
                pend_v.append((dA_t, hh_pair))
                ch_defer.append((hh_pair, i))
                if i == CBL:
                    # post-AR path, emitted after CBL iterations of eager
                    # work: only the deferred mixed flushes below (and the
                    # final vt scaling) consume the AllReduce result
                    mden = cpool.tile([128, 4], f32, name="mden")
                    nc.gpsimd.tensor_tensor(mden[:], den2b[:], bmask_sb[:],
                                            AL.mult)
                    myden = cpool.tile([128, 2], f32, name="myden")
                    nc.gpsimd.tensor_tensor(myden[:], mden[:, 0:2],
                                            mden[:, 2:4], AL.add)
                    for j in range(2):
                        nc.gpsimd.normalize_recip(cb[:, j * TC:(j + 1) * TC],
                                                  cbraw[:, j * TC:(j + 1) * TC],
                                                  myden[:, j:j + 1])
                if i >= CBL:
                    flush_mixed()

            load_wd(0, tag="xw", bufs=2)
            for di in range(1, 6):
                load_wd(di)
            flush_v(last=True)
            # masked v with the deferred routing weight, folded across slots:
            # down_B is the same for both slots, so dB@vt0 + dB@vt1 ==
            # dB@(vt0+vt1) -- one matmul per d-tile instead of two
            vm = spool.tile([128, 2 * TC], f32, tag="vm", bufs=1)
            for j in range(2):
                nc.vector.tensor_tensor(vm[:, j * TC:(j + 1) * TC],
                                        psV[:, j * TC:(j + 1) * TC],
                                        Mj[:, j * TC:(j + 1) * TC], AL.mult)
                nc.vector.tensor_tensor(vt[:, j * TC:(j + 1) * TC],
                                        vm[:, j * TC:(j + 1) * TC],
                                        cb[:, j * TC:(j + 1) * TC], AL.mult)
            vsum = spool.tile([128, TC], fp16, tag="vm", bufs=1)
            nc.vector.tensor_tensor(vsum[:], vt[:, 0:TC], vt[:, TC:2 * TC],
                                    AL.add)
            while ch_defer:
                flush_mixed(drain=True)

            if DEBUG_TAPS:
                for nm, tl, dt_ in [("cbraw", cbraw, f32), ("cb", cb, f32),
                                    ("Mj", Mj, f32), ("UA", UA, f32),
                                    ("vt", vt, f32)]:
                    shp = [tl.shape[0], tl.shape[-1]]
                    dbg = nc.dram_tensor(f"dbg_{nm}", shp, f32,
                                         kind="ExternalOutput").ap()
                    nc.sync.dma_start(out=dbg[:], in_=tl[:])

            # ---- phase 7: down GEMM (fp8 hi/lo 3-chain DR) + LoRA-down ----
            HT2 = HT // 2
            for di in range(DT):
                if di not in wd_pre:
                    load_wd(di)
                if di + 6 < DT and di + 6 not in wd_pre:
                    load_wd(di + 6)
                wdh_t, wdl_t = wd_pre.pop(di)
                psO = pspool.tile([128, TC], f32, tag="psUG", bufs=3, name="psO")
                chains = [(wdh_t, mixed_h8), (wdl_t, mixed_h8),
                          (wdh_t, mixed_l8)]
                for ci, (wt, mt) in enumerate(chains):
                    for m2 in range(HT2):
                        nc.tensor.matmul(
                            psO[:], wt[:, 2 * m2:2 * m2 + 2, :],
                            mt[:, 2 * m2:2 * m2 + 2, :],
                            start=(ci == 0 and m2 == 0), stop=False,
                            perf_mode=DR, skip_group_check=True)
                nc.tensor.matmul(psO[:], dB_sb[:, di * 128:(di + 1) * 128],
                                 vsum[:], start=False, stop=True,
                                 skip_group_check=True)
                o_sb = spool.tile([128, TC], bf16, tag="o_sb")
                nc.scalar.activation(o_sb[:], psO[:], AF.Copy,
                                     scale=1.0 / (SS * SWD))
                nc.sync.dma_start(out=d_out[di * 128:(di + 1) * 128, :],
                                  in_=o_sb[:])

    nc.compile()
    return nc


def _prep_shared(inputs):
    """Host-side layout prep of weight tensors (shared across cores)."""
    import ml_dtypes
    bf16 = np.dtype(ml_dtypes.bfloat16)
    fp16 = np.float16
    fp8 = np.dtype(ml_dtypes.float8_e4m3)
    f32 = np.float32

    def c(a, dt):
        return np.ascontiguousarray(a.astype(dt, copy=False))

    def split8(a, s):
        hi = (a * s).astype(fp8)
        lo = ((a * s) - hi.astype(f32)).astype(fp8)
        return hi, lo

    w_up, w_gate, w_down = (inputs[k].astype(f32) for k in
                            ("w_up", "w_gate", "w_down"))
    # per h-tile [128(k), KT, 128(h)] layout, hi | lo stacked on the KT axis
    def wpack(w, s):                       # w [H, D]
        hi, lo = split8(w, s)
        def lay(a):
            return (a.astype(f32).reshape(HT, 128, KT, 128)
                    .transpose(0, 3, 2, 1))        # [HT, 128k, KT, 128h]
        return c(np.concatenate([lay(hi), lay(lo)], axis=2), fp8)

    wu = wpack(w_up, SWU)
    wg = wpack(w_gate, SWU)

    wdh_, wdl_ = split8(w_down, SWD)       # [D, H]
    def wdlay(a):                          # -> [DT, 128(h), HT, 128(d)]
        return c(a.astype(f32).reshape(DT, 128, HT, 128)
                 .transpose(0, 3, 2, 1), fp8)
    wdh = wdlay(wdh_)
    wdl = wdlay(wdl_)

    # Ah cols 0:KT*ER = up_A (k-major), KT*ER: = gate_A
    def a_half(a):
        return (a.reshape(ER, D).reshape(ER, KT, 128).transpose(2, 1, 0)
                .reshape(128, KT * ER))
    Ah = c(np.concatenate([a_half(inputs["up_A"]),
                           a_half(inputs["gate_A"])], axis=1), bf16)

    up_B_all = (inputs["up_B"].transpose(0, 2, 1).reshape(ER, H)
                * (ALPHA * SB)).astype(f32)
    gate_B_all = (inputs["gate_B"].transpose(0, 2, 1).reshape(ER, H)
                  * (ALPHA * SB)).astype(f32)
    uB8 = up_B_all.reshape(ER, HT, 128).transpose(1, 0, 2).astype(fp8)
    gB8 = gate_B_all.reshape(ER, HT, 128).transpose(1, 0, 2).astype(fp8)

    down_A_all = inputs["down_A"].reshape(ER, H).astype(f32)
    dA16 = down_A_all.T.reshape(HT, 128, ER).astype(fp16)
    # one [HT, 128, 512] fp8-byte pack: up_B8 | gate_B8 | down_A(fp16 bytes)
    u8 = np.uint8
    Bp = np.ascontiguousarray(np.concatenate(
        [np.ascontiguousarray(uB8).view(u8),
         np.ascontiguousarray(gB8).view(u8),
         np.ascontiguousarray(dA16).view(u8)], axis=2)).view(fp8)
    down_B_all = (inputs["down_B"].transpose(0, 2, 1).reshape(ER, D)
                  * (ALPHA * SWD)).astype(f32)
    dB = c(down_B_all, fp16)

    gate_wT = inputs["gate_w"].T.astype(f32)               # [D, E]
    gwf = (gate_wT.reshape(KT, 128, E).transpose(1, 0, 2)
           .reshape(128, KT * E))
    gwh = gwf.astype(bf16)
    gwl = (gwf - gwh.astype(f32)).astype(bf16)
    gwp = c(np.concatenate([gwh, gwl], axis=1), bf16)

    eid = (8.0 - (np.arange(128) // R)).astype(f32).reshape(128, 1)
    i8m = np.tile((8.0 - np.arange(E)).astype(f32), (128, 1))
    ei = c(np.concatenate([eid, i8m], axis=1), f32)
    sel2 = np.zeros((2, 256), f32)
    sel2[0, 0:128] = 1.0
    sel2[1, 128:256] = 1.0

    return dict(wu=wu, wg=wg, wdh=wdh, wdl=wdl, Ah=Ah, Bp=Bp, dB=dB,
                gwp=gwp, ei=ei, sel2=sel2)


def kernel(**inputs):
    import ml_dtypes
    from concourse.bass_utils import run_bass_kernel_spmd

    bf16 = np.dtype(ml_dtypes.bfloat16)
    fp8 = np.dtype(ml_dtypes.float8_e4m3)
    inputs = {k: np.asarray(v) for k, v in inputs.items()}
    if "nc" not in _cache:
        _cache["nc"] = _build()
    nc = _cache["nc"]

    shared = _prep_shared(inputs)
    x = inputs["x"].astype(np.float32)
    xt = x.reshape(T, D)

    in_maps = []
    for cix in range(NCORES):
        xc = xt[cix * TC:(cix + 1) * TC]                   # [TC, D]
        xT = xc.T                                          # [D, TC] f32
        # sb layout [128, KT*TC]: sb[p, k*TC+t] = x[k*128+p, t]
        xTs = np.ascontiguousarray(
            xT.reshape(KT, 128, TC).transpose(1, 0, 2).reshape(128, KT * TC))
        b = (cix * TC) // S
        bscat = np.zeros((2, 4), np.float32)
        bscat[0, 2 * b] = 1.0
        bscat[1, 2 * b + 1] = 1.0
        bmask = np.zeros((128, 4), np.float32)
        bmask[:, 2 * b:2 * b + 2] = 1.0
        xhi = xTs.astype(bf16)
        xl = (xTs - xhi.astype(np.float32)).astype(bf16)
        xs8 = xTs * SX
        xh8 = xs8.astype(fp8)
        xl8 = (xs8 - xh8.astype(np.float32)).astype(fp8)
        m = dict(shared)
        m["xTb"] = np.ascontiguousarray(xhi)
        m["xlo"] = np.ascontiguousarray(xl)
        m["xh8"] = np.ascontiguousarray(xh8.reshape(128, KT, TC))
        m["xl8"] = np.ascontiguousarray(xl8.reshape(128, KT, TC))
        m["bscat"] = bscat
        m["bmask"] = bmask
        in_maps.append(m)

    res = run_bass_kernel_spmd(nc, in_maps, list(range(NCORES)))
    out = np.empty((T, D), np.float32)
    for cix in range(NCORES):
        out[cix * TC:(cix + 1) * TC, :] = res.results[cix]["outT"].T
    return out.reshape(B, S, D)
